# revision 43
# baseline (speedup 1.0000x reference)
"""Additive (Bahdanau) attention on 4 of 8 TRN2 NeuronCores.

Problem shapes: B=4, Q=512, K=1024, Dq=Dk=Dv=512, H=128.

Sharding: one batch per core on a 4-core mesh (cores 4-7 idle). The
metric this kernel is tuned for is the MARGINAL WALL-CLOCK PER DISPATCH
through the axon tunnel, and that cost scales with mesh size (~100 us
fixed + ~15-30 us per core: measured sustained slopes for a trivial
NEFF are 99/92/116/233 us at 1/2/4/8 cores), while the on-device time
scales down with more cores. 8 cores put the device at ~72 us but
dispatch at ~233 us; 4 cores put dispatch at ~116 us and the device at
~95-115 us (each core shares ALL key-side work -- k features, k trig,
values cast -- across its two query halves, so 4-core device time is
well under 2x the 8-core time). max(dispatch, device) is minimized at
4 cores.

Algorithm (sine decomposition of additive attention):

tanh(a+b) is separable through the angle-addition identity. Fit
tanh(x) ~ sum_r c_r sin(w_r x) (weighted least squares, R=8, wmax=4.5,
core max err ~2e-3 over the feature-sum range), then

  scores[q,k] = sum_h w_h tanh(qf_hq + kf_hk)
             = sum_r [ (c_r w_h sin(w_r qf)) . cos(w_r kf)
                     + (c_r w_h cos(w_r qf)) . sin(w_r kf) ]   (contract h)

i.e. 2R=16 accumulating 128-contraction matmuls on the tensor engine
instead of 268M scalar-engine tanh evaluations (~300 us/core direct).
The ACT Sin table is only accurate on [-pi, pi], so arguments are
range-reduced exactly:

  t   = x * (w/2pi)                 (DVE tensor_scalar; ACT Copy q-side)
  a_s = fl(t + 1.5*2^23)            (ACT Copy with float bias: the fp32
                                     store rounds t to the nearest
                                     integer; bit-identical to the DVE
                                     tensor_scalar version)
  e_s = (a_s - 1.5*2^23) - t        (DVE scalar_tensor_tensor; both
                                     terms exact) = round(t) - t
  sin(w x) = sin(-2pi * e_s)        (ACT Sin, scale = -2pi)
  e_c  = wrap(e_s - 1/4)            (one DVE add_range_wrap into
                                     [-1/2, 1/2]; -2pi*e_c = -2pi*e_s
                                     + pi/2 (mod 2pi))
  cos(w x) = sin(-2pi * e_c)        (same ACT Sin, no bias -- the whole
                                     second reduction chain collapses
                                     into one custom-DVE op)

IMPORTANT: no GPSIMD (Q7) instructions anywhere -- each dispatch of a
NEFF containing GPSIMD ops costs ~300-400 us of extra per-dispatch
host/runtime overhead under axon, dwarfing the on-device time. The
trig chains are balanced across DVE and ACT only; k-side ops are fused
1024 wide. sin/cos tiles are bf16 (the c_r*w_h weighting is folded
into the q-side, keeping per-term magnitudes small). Inputs arrive as
ONE packed DRAM parameter (fewer PJRT buffer binds per dispatch).

Score accumulation note: matmul start=True clears the has_written bits
of the whole PSUM bank, so interleaved accumulation groups sharing a
bank cannot use it. A dummy full-bank start=True matmul writes zeros
and sets every bit; the real matmuls then accumulate with start=False
in r-streaming order. Scores for the two query halves go through the
same 4-bank PSUM region sequentially (8 banks total: 4 scores + 2x2
output/denominator); the tile framework's WAR tracking makes half B's
zeroing wait for half A's exp reads, and the PE fills that bubble with
half A's attn@V matmuls.

Softmax needs no max-subtraction (scores are O(1) by construction: w_v
has variance 1/H). exp runs once over the transposed scores [k, q] in
PSUM so the attn tile is directly the stationary operand of the attn@V
matmuls; the softmax denominator comes from one extra accumulating
matmul against a ones vector, followed by a DVE reciprocal and a
per-partition rescale of the output.

Dispatch-path notes (the dominant cost): the runner compiles under
bass2jax._fast_dispatch_active(True) so _bass_exec_p declares no
effect and calls take jax's C++ pjit fast path (the Python
effects/token path costs ~0.3-0.7 ms/call more); it returns the raw
Compiled rather than the FastDispatchCompiled wrapper (whose per-call
Python re-registration of every output shard costs ~0.1-0.2 ms); and
it passes no zero-filled output operands (on the exec lowering path
the NEFF binds only BIR ExternalInputs, and this kernel DMA-writes
every element of out). partition_id is disabled (unused input).
"""


import os
import numpy as np

import concourse.bass as bass
import concourse.mybir as mybir
import concourse.tile as tile
from concourse import bacc
from concourse.bass_utils import run_bass_kernel_spmd
from concourse.masks import make_identity

B, Q, K, D, H = 4, 512, 1024, 512, 128
N_CORES = 4
QSH = Q                         # 512 query rows per core (one batch/core)
QH = 256                        # query rows per PSUM score pass
NDC = D // 128                  # 4 contraction chunks
NKC = K // 128                  # 8 key chunks
NQB = QSH // 128                # 4 query blocks per core

F32 = mybir.dt.float32
BF16 = mybir.dt.bfloat16
EXP = mybir.ActivationFunctionType.Exp
SIN = mybir.ActivationFunctionType.Sin
COPY = mybir.ActivationFunctionType.Copy
TS = mybir.AluOpType

MAGIC = 12582912.0              # 1.5 * 2**23: fp32 add forces round-to-int
TWO_PI = float(2.0 * np.pi)

LAST_EXEC_NS = None
_NC_CACHE = {}


R_SINE = 8
WMAX = 4.5

def _fit_sine(R=R_SINE, wmax=WMAX, L=8.5, sigma=1.7):
    """Least-squares fit tanh(x) ~ sum_r c_r sin(w_r x) on [-L, L]."""
    ws = np.linspace(wmax / R * 0.5, wmax, R)
    xs = np.linspace(-L, L, 4001)
    wt = np.exp(-xs ** 2 / (2 * sigma ** 2)) + 1e-3
    A = np.sin(np.outer(xs, ws))
    Wt = np.sqrt(wt)[:, None]
    c, *_ = np.linalg.lstsq(A * Wt, np.tanh(xs) * Wt[:, 0], rcond=None)
    return [float(w) for w in ws], [float(v) for v in c]


# Packed single-input layout (one NEFF parameter instead of six: fewer PJRT
# buffer binds per dispatch, which dominates the per-call overhead under axon).
OFF_Q = 0
OFF_K = OFF_Q + QSH * D          # 262144
OFF_V = OFF_K + K * D            # 786432
OFF_WQ = OFF_V + K * D           # 1310720
OFF_WK = OFF_WQ + D * H          # 1376256
OFF_WV = OFF_WK + D * H          # 1441792
NPACK = OFF_WV + H               # 1441920


def _declare_io(nc):
    xin = nc.declare_dram_parameter("xin", [NPACK], F32, isOutput=False)
    out_ext = nc.declare_dram_parameter("out", [QSH, D], F32, isOutput=True)
    q_ext = xin[OFF_Q:OFF_K].rearrange("(q d) -> q d", d=D)
    k_ext = xin[OFF_K:OFF_V].rearrange("(k d) -> k d", d=D)
    v_ext = xin[OFF_V:OFF_WQ].rearrange("(k d) -> k d", d=D)
    wq_ext = xin[OFF_WQ:OFF_WK].rearrange("(d h) -> d h", h=H)
    wk_ext = xin[OFF_WK:OFF_WV].rearrange("(d h) -> d h", h=H)
    wv_ext = xin[OFF_WV:NPACK].rearrange("(h o) -> h o", o=1)
    return q_ext, k_ext, v_ext, wq_ext, wk_ext, wv_ext, out_ext


def _preamble(nc, tc, const, work, feat, q_ext, k_ext, v_ext, wq_ext, wk_ext,
              wv_ext):
    """DMA + PE-transpose inputs, feature matmuls, values cast.

    DMA instructions are BATCHED (one strided DMA per tensor / k-half
    instead of one per 128-row tile): each DMACopy costs ~0.6-1.2 us of
    sequencer issue plus ~0.9 us of completion-semaphore propagation, so
    27 small DMAs were a two-digit-us tax on the SEQ pipelines. All loads
    go on the SP (sync) queue; the ACT queue issues no DMAs so its
    sequencer bandwidth stays on activations. Keys load FIRST (in two
    512-row halves so the kf features and the k-side trig -- the long
    pole -- start as early as possible), then queries, then values
    (deferred: nothing reads v until the attn@V tail).

    Returns (fq [H, QSH+K] f32 ([qf | kf]), v_b [128, NKC, D] bf16,
    wv_f [H,1] f32)."""
    ident = const.tile([128, 128], F32)
    make_identity(nc, ident)

    wq_t = const.tile([128, NDC, H], F32)
    wk_t = const.tile([128, NDC, H], F32)
    nc.sync.dma_start(out=wk_t, in_=wk_ext.rearrange("(c p) h -> p c h", p=128))
    nc.sync.dma_start(out=wq_t, in_=wq_ext.rearrange("(c p) h -> p c h", p=128))

    wv_f = const.tile([H, 1], F32)
    nc.sync.dma_start(out=wv_f, in_=wv_ext[:])

    k_all = feat.tile([128, NKC, D], F32)
    for hf in range(2):
        nc.sync.dma_start(
            out=k_all[:, hf * 4:(hf + 1) * 4, :],
            in_=k_ext[hf * 512:(hf + 1) * 512, :].rearrange(
                "(c p) d -> p c d", p=128))
    q_all = feat.tile([128, QSH // 128, D], F32)
    nc.sync.dma_start(out=q_all,
                      in_=q_ext.rearrange("(c p) d -> p c d", p=128))

    qT = feat.tile([128, NDC, QSH], F32)
    kT = feat.tile([128, NDC, K], F32)
    # fq = [qf | kf] in ONE tile so the per-r trig argument prep can run as
    # single wide DVE ops over both sides at once.
    fq = feat.tile([H, QSH + K], F32)
    qf_sb = fq[:, :QSH]
    kf_sb = fq[:, QSH:]
    with tc.tile_pool(name="pre_ps", bufs=4, space="PSUM") as pre_ps:
        for t in range(K // 128):
            for dc in range(NDC):
                tp = pre_ps.tile([128, 128], F32, tag="tps")
                nc.tensor.transpose(tp, k_all[:, t, dc * 128:(dc + 1) * 128],
                                    ident)
                dst = kT[:, dc, t * 128:(t + 1) * 128]
                (nc.vector.tensor_copy(dst, tp) if dc % 2 == 0
                 else nc.scalar.copy(dst, tp))
            if t == 3 or t == 7:
                hf = t // 4
                kf_ps = pre_ps.tile([H, 512], F32, tag="fps")
                for dc in range(NDC):
                    nc.tensor.matmul(kf_ps, wk_t[:, dc, :],
                                     kT[:, dc, hf * 512:(hf + 1) * 512],
                                     start=(dc == 0), stop=(dc == NDC - 1))
                dst = kf_sb[:, hf * 512:(hf + 1) * 512]
                (nc.vector.tensor_copy(dst, kf_ps) if hf == 0
                 else nc.scalar.copy(dst, kf_ps))

        for t in range(QSH // 128):
            for dc in range(NDC):
                tp = pre_ps.tile([128, 128], F32, tag="tps")
                nc.tensor.transpose(tp, q_all[:, t, dc * 128:(dc + 1) * 128],
                                    ident)
                dst = qT[:, dc, t * 128:(t + 1) * 128]
                (nc.vector.tensor_copy(dst, tp) if dc % 2 == 0
                 else nc.scalar.copy(dst, tp))

        qf_ps = pre_ps.tile([H, QSH], F32, tag="fps")
        for dc in range(NDC):
            nc.tensor.matmul(qf_ps, wq_t[:, dc, :], qT[:, dc, :],
                             start=(dc == 0), stop=(dc == NDC - 1))
        nc.vector.tensor_copy(qf_sb, qf_ps)

    v_b = feat.tile([128, NKC, D], BF16)
    tc.tile_set_cur_wait(0.05)   # keep values off the keys->kf critical path
    v_stage = feat.tile([128, NKC, D], F32)
    nc.sync.dma_start(out=v_stage,
                      in_=v_ext.rearrange("(c p) d -> p c d", p=128))
    for hf in range(2):
        (nc.vector.tensor_copy if hf else nc.scalar.copy)(
            v_b[:, hf * 4:(hf + 1) * 4, :], v_stage[:, hf * 4:(hf + 1) * 4, :])
    tc.tile_set_cur_wait(0)

    return fq, v_b, wv_f


def _build_sine():
    ws, cs = _fit_sine()
    R = len(ws)
    nc = bacc.Bacc(enable_partition_id=False)
    q_ext, k_ext, v_ext, wq_ext, wk_ext, wv_ext, out_ext = _declare_io(nc)

    with tile.TileContext(nc) as tc:
        with tc.tile_pool(name="const", bufs=1) as const, \
             tc.tile_pool(name="work", bufs=3) as work, \
             tc.tile_pool(name="feat", bufs=1) as feat, \
             tc.tile_pool(name="trig", bufs=2) as trig, \
             tc.tile_pool(name="oloop", bufs=2) as oloop:

            fq, v_b, wv_f = _preamble(
                nc, tc, const, work, feat, q_ext, k_ext, v_ext,
                wq_ext, wk_ext, wv_ext)

            # per-r q-side coefficient vectors: wc[:, r] = c_r * w_v
            wc = const.tile([H, R], F32)
            for r in range(R):
                nc.vector.tensor_scalar_mul(wc[:, r:r + 1], wv_f, float(cs[r]))

            W2 = QSH + K                 # one trig block: [q(512) | k(1024)]
            # SCB[r] = [wc*sin_q(512) | sin_k(1024) |
            #           wc*cos_q(512) | cos_k(1024)]
            # One tile PER r (not [H, R, 2*W2]): the PE matmul AP encodes a
            # per-partition stride that maxes out at 16 KiB, and the fused
            # tile's 24 KiB row fails walrus's s3d3_mm ISA check.
            SCB = [feat.tile([H, 2 * W2], BF16, name=f"scb{r}")
                   for r in range(R)]

            # Per r, the range reduction is 3 wide DVE ops + 1 ACT round +
            # ONE double-wide ACT Sin over the combined [qf | kf] block:
            #   t   = x * (w/2pi)                  (DVE tensor_scalar)
            #   a   = fl(t + 1.5*2^23)             (ACT Copy, float bias:
            #                                       the fp32 store rounds t
            #                                       to the nearest integer)
            #   e_s = (a - MAGIC) - t              (DVE scalar_tensor_tensor,
            #                                       exact) = round(t) - t
            #   e_c = wrap(e_s - 1/4)              (custom-DVE range wrap
            #                                       into [-1/2, 1/2])
            #   sin/cos(w x) = Sin(-2pi * e)       (ONE ACT Sin over the
            #                                       contiguous [e_s | e_c]
            #                                       tile; table is accurate
            #                                       on [-pi, pi])
            # The q-side c_r*w_v weighting is applied in place on the bf16
            # sin/cos q slices (2 narrow DVE muls). No GPSIMD (Q7) anywhere:
            # those cost ~300us of per-dispatch host overhead under axon.
            for r in range(R):
                w2p = float(ws[r] / TWO_PI)
                wcol = wc[:, r:r + 1]
                t_t = trig.tile([H, W2], F32, tag="t")
                nc.vector.tensor_scalar(t_t, fq, w2p, None, TS.mult)
                a_t = trig.tile([H, W2], F32, tag="a")
                nc.scalar.activation(out=a_t, in_=t_t, func=COPY, bias=MAGIC)
                arg = trig.tile([H, 2 * W2], F32, tag="arg")
                nc.vector.scalar_tensor_tensor(arg[:, :W2], a_t, MAGIC, t_t,
                                               TS.subtract, TS.subtract)
                nc.vector.add_range_wrap(arg[:, W2:], arg[:, :W2],
                                         -0.25, 0.5, 1.0)
                nc.scalar.activation(out=SCB[r][:], in_=arg,
                                     func=SIN, scale=-TWO_PI)
                nc.vector.tensor_scalar_mul(SCB[r][:, 0:QSH],
                                            SCB[r][:, 0:QSH], wcol)
                nc.vector.tensor_scalar_mul(SCB[r][:, W2:W2 + QSH],
                                            SCB[r][:, W2:W2 + QSH], wcol)


            # Scores in [q, k] orientation: per 128-query block, stationary
            # = QS/QC q-chunk [H, 128] and moving = the FULL 1024-wide
            # KS/KC row, so one PSUM pass is 2R=16 matmuls of 1024 moving
            # cols instead of 128 matmuls of 256 cols. This unloads the PE
            # SEQUENCER (Ldweights+Matmult issue was the critical path at
            # 256 score matmuls). Each [128, 1024] f32 region is exactly 2
            # PSUM banks used by a single accumulation group, so plain
            # start/stop works -- no dummy-zero matmuls.
            #
            # attn is then transposed for the attn@V matmuls by ONE xbar
            # DMA-transpose per block (64 16x128 tiles, ~1 us on the idle
            # DMA engines) instead of 8 PE transposes + 8 PSUM copies. The
            # xbar writes logical row r of attn^T to attnT[r % 128,
            # r // 128, :] (hardware-verified), i.e. k-chunk c holds k rows
            # {c*128+p} -- the natural chunk layout v_b is loaded in.
            # Score matmuls for block qb+1 are issued BEFORE block qb's
            # transpose/AV so the PE never stalls on qb's exp.
            o_all = feat.tile([128, NQB, D], F32)
            with tc.tile_pool(name="psqk", bufs=3, space="PSUM") as psqk, \
                 tc.tile_pool(name="ps", bufs=2, space="PSUM") as ps:
                qsc = [None] * NQB

                def issue_scores(qb):
                    qs_sl = slice(qb * 128, (qb + 1) * 128)
                    qc_sl = slice(W2 + qb * 128, W2 + (qb + 1) * 128)
                    ks_sl = slice(QSH, W2)
                    kc_sl = slice(W2 + QSH, 2 * W2)
                    qsc[qb] = psqk.tile([128, K], F32, tag="qsc",
                                        name=f"qsc{qb}")
                    # moving operands are split into 512-wide halves: a
                    # 1024-element moving fmap fails walrus's s3d3_mm ISA
                    # check. The stationary is identical for both halves, so
                    # the second matmul skips its Ldweights.
                    for r in range(R):
                        for hf in range(2):
                            osl = slice(hf * 512, (hf + 1) * 512)
                            nc.tensor.matmul(
                                qsc[qb][:, osl], SCB[r][:, qs_sl],
                                SCB[r][:, kc_sl][:, osl],
                                start=(r == 0), stop=False,
                                skip_group_check=True)
                        for hf in range(2):
                            osl = slice(hf * 512, (hf + 1) * 512)
                            nc.tensor.matmul(
                                qsc[qb][:, osl], SCB[r][:, qc_sl],
                                SCB[r][:, ks_sl][:, osl],
                                start=False, stop=(r == R - 1),
                                skip_group_check=True)

                issue_scores(0)
                issue_scores(1)
                for qb in range(NQB):
                    attnQ = oloop.tile([128, K], BF16, tag="attnQ")
                    d_sb = oloop.tile([128, 1], F32, tag="dsb")
                    # accum_out gives the softmax denominator for free: in
                    # the [q, k] orientation the activation's per-partition
                    # output sum IS sum_k exp(score[q, k]).
                    nc.scalar.activation(out=attnQ, in_=qsc[qb], func=EXP,
                                         accum_out=d_sb)
                    if qb + 2 < NQB:
                        issue_scores(qb + 2)
                    attnT = oloop.tile([128, NKC, 128], BF16, tag="attnT")
                    nc.sync.dma_start_transpose(out=attnT[:], in_=attnQ[:])
                    o_ps = ps.tile([128, D], F32, tag="ops")
                    for kc in range(NKC):
                        nc.tensor.matmul(o_ps, attnT[:, kc, :], v_b[:, kc, :],
                                         start=(kc == 0), stop=(kc == NKC - 1))
                    recip = oloop.tile([128, 1], F32, tag="recip")
                    nc.vector.reciprocal(recip, d_sb)
                    nc.vector.tensor_scalar_mul(o_all[:, qb, :], o_ps, recip)
                nc.sync.dma_start(
                    out=out_ext.rearrange("(t p) d -> p t d", p=128),
                    in_=o_all)
    nc.compile()
    return nc


def _get_nc():
    if "sine" not in _NC_CACHE:
        _NC_CACHE["sine"] = _build_sine()
    return _NC_CACHE["sine"]


def make_in_maps(queries, keys, values, W_q, W_k, w_v):
    queries = np.asarray(queries, dtype=np.float32)
    keys = np.asarray(keys, dtype=np.float32)
    values = np.asarray(values, dtype=np.float32)
    W_q = np.asarray(W_q, dtype=np.float32).ravel()
    W_k = np.asarray(W_k, dtype=np.float32).ravel()
    w_v = np.asarray(w_v, dtype=np.float32).ravel()
    in_maps = []
    for c in range(N_CORES):
        buf = np.empty(NPACK, np.float32)
        buf[OFF_Q:OFF_K] = queries[c].ravel()
        buf[OFF_K:OFF_V] = keys[c].ravel()
        buf[OFF_V:OFF_WQ] = values[c].ravel()
        buf[OFF_WQ:OFF_WK] = W_q
        buf[OFF_WK:OFF_WV] = W_k
        buf[OFF_WV:NPACK] = w_v
        in_maps.append({"xin": buf})
    return in_maps


_RUNNER_CACHE = {}


def _get_runner(nc):
    """Persistent compiled shard_map runner for nc (compiled once/process).

    Two dispatch-path choices matter for the marginal per-call cost under
    axon (the per-dispatch host overhead dominates on-device time):

    * compile under bass2jax._fast_dispatch_active(True): _bass_exec_p then
      declares no effect, so calls take jax's C++ pjit fast path instead of
      the Python effects/token dispatch (~0.3-0.7 ms/call cheaper).
    * return the raw Compiled, NOT FastDispatchCompiled: the safety-net
      wrapper re-registers every output shard in runtime_tokens on every
      call (a Python loop over the shards, ~0.1-0.2 ms/call). kernel()
      reads its outputs immediately, so device errors surface regardless.
    * no zero-filled output operands: on the exec lowering path the NEFF
      binds only BIR ExternalInputs (the "out" zeros param has no NEFF
      tensor and is ignored), and this kernel DMA-writes every element of
      out, so PJRT's uninitialized result allocation is fine. Dropping
      them saves one buffer bind per core per call.
    """
    if id(nc) in _RUNNER_CACHE:
        return _RUNNER_CACHE[id(nc)]
    import jax
    from jax.sharding import Mesh, NamedSharding, PartitionSpec
    from jax.experimental.shard_map import shard_map
    from concourse import bass2jax

    bass2jax.install_neuronx_cc_hook()
    partition_name = (nc.partition_id_tensor.name
                      if nc.partition_id_tensor else None)
    in_names, in_shapes, out_names, out_avals = [], [], [], []
    for alloc in nc.m.functions[0].allocations:
        if not isinstance(alloc, mybir.MemoryLocationSet):
            continue
        name = alloc.memorylocations[0].name
        if alloc.kind == "ExternalInput":
            if name != partition_name:
                in_names.append(name)
                in_shapes.append(
                    (tuple(alloc.tensor_shape), mybir.dt.np(alloc.dtype)))
        elif alloc.kind == "ExternalOutput":
            out_names.append(name)
            shape = tuple(alloc.tensor_shape)
            dtype = mybir.dt.np(alloc.dtype)
            out_avals.append(jax.core.ShapedArray(shape, dtype))
    all_in_names = list(in_names)
    if partition_name is not None:
        all_in_names.append(partition_name)

    def _body(*args):
        operands = list(args)
        if partition_name is not None:
            operands.append(bass2jax.partition_id_tensor())
        outs = bass2jax._bass_exec_p.bind(
            *operands,
            out_avals=tuple(out_avals),
            in_names=tuple(all_in_names),
            out_names=tuple(out_names),
            lowering_input_output_aliases=(),
            sim_require_finite=True,
            sim_require_nnan=True,
            nc=nc,
        )
        return tuple(outs)

    devices = jax.devices()[:N_CORES]
    mesh = Mesh(np.asarray(devices), ("core",))
    nio = len(in_names)
    sharding = NamedSharding(mesh, PartitionSpec("core"))
    fast_ctx = getattr(bass2jax, "_fast_dispatch_active", None)
    import contextlib
    with (fast_ctx(True) if fast_ctx is not None
          else contextlib.nullcontext()):
        f = jax.jit(
            shard_map(_body, mesh=mesh,
                      in_specs=(PartitionSpec("core"),) * nio,
                      out_specs=(PartitionSpec("core"),) * len(out_names),
                      check_rep=False),
            keep_unused=True,
        )
        dummy_in = [
            jax.ShapeDtypeStruct((N_CORES * shape[0], *shape[1:]), dtype,
                                 sharding=sharding)
            for shape, dtype in in_shapes
        ]
        fc = f.lower(*dummy_in).compile()
    runner = (fc, in_names, out_names, out_avals, sharding)
    _RUNNER_CACHE[id(nc)] = runner
    return runner


def kernel(queries, keys, values, W_q, W_k, w_v):
    import jax
    nc = _get_nc()
    in_maps = make_in_maps(queries, keys, values, W_q, W_k, w_v)
    try:
        fc, in_names, out_names, out_avals, sharding = _get_runner(nc)
        concat_in = [
            np.concatenate([in_maps[c][name] for c in range(N_CORES)], axis=0)
            for name in in_names
        ]
        args = [jax.device_put(a, sharding) for a in concat_in]
        out_arrs = fc(*args)
        results = [
            {name: np.asarray(out_arrs[i]).reshape(
                N_CORES, *out_avals[i].shape)[c]
             for i, name in enumerate(out_names)}
            for c in range(N_CORES)
        ]
    except Exception:
        res = run_bass_kernel_spmd(nc, in_maps, core_ids=list(range(N_CORES)))
        results = res.results

    out = np.empty((B, Q, D), dtype=np.float32)
    for c in range(N_CORES):
        out[c] = results[c]["out"]
    return out


# revision 49
# speedup vs baseline: 1.1187x; 1.1187x over previous
"""Additive (Bahdanau) attention on 4 of 8 TRN2 NeuronCores.

Problem shapes: B=4, Q=512, K=1024, Dq=Dk=Dv=512, H=128.

Sharding: one batch per core on a 4-core mesh (cores 4-7 idle). The
metric this kernel is tuned for is the MARGINAL WALL-CLOCK PER DISPATCH
through the axon tunnel, and that cost scales with mesh size (~100 us
fixed + ~15-30 us per core: measured sustained slopes for a trivial
NEFF are 99/92/116/233 us at 1/2/4/8 cores), while the on-device time
scales down with more cores. 8 cores put the device at ~72 us but
dispatch at ~233 us; 4 cores put dispatch at ~116 us and the device at
~95-115 us (each core shares ALL key-side work -- k features, k trig,
values cast -- across its two query halves, so 4-core device time is
well under 2x the 8-core time). max(dispatch, device) is minimized at
4 cores.

Algorithm (sine decomposition of additive attention):

tanh(a+b) is separable through the angle-addition identity. Fit
tanh(x) ~ sum_r c_r sin(w_r x) (weighted least squares, R=8, wmax=4.5,
core max err ~2e-3 over the feature-sum range), then

  scores[q,k] = sum_h w_h tanh(qf_hq + kf_hk)
             = sum_r [ (c_r w_h sin(w_r qf)) . cos(w_r kf)
                     + (c_r w_h cos(w_r qf)) . sin(w_r kf) ]   (contract h)

i.e. 2R=16 accumulating 128-contraction matmuls on the tensor engine
instead of 268M scalar-engine tanh evaluations (~300 us/core direct).
The ACT Sin table is only accurate on [-pi, pi], so arguments are
range-reduced exactly:

  t   = x * (w/2pi)                 (DVE tensor_scalar; ACT Copy q-side)
  a_s = fl(t + 1.5*2^23)            (ACT Copy with float bias: the fp32
                                     store rounds t to the nearest
                                     integer; bit-identical to the DVE
                                     tensor_scalar version)
  e_s = (a_s - 1.5*2^23) - t        (DVE scalar_tensor_tensor; both
                                     terms exact) = round(t) - t
  sin(w x) = sin(-2pi * e_s)        (ACT Sin, scale = -2pi)
  e_c  = wrap(e_s - 1/4)            (one DVE add_range_wrap into
                                     [-1/2, 1/2]; -2pi*e_c = -2pi*e_s
                                     + pi/2 (mod 2pi))
  cos(w x) = sin(-2pi * e_c)        (same ACT Sin, no bias -- the whole
                                     second reduction chain collapses
                                     into one custom-DVE op)

IMPORTANT: no GPSIMD (Q7) instructions anywhere -- each dispatch of a
NEFF containing GPSIMD ops costs ~300-400 us of extra per-dispatch
host/runtime overhead under axon, dwarfing the on-device time. The
trig chains are balanced across DVE and ACT only; k-side ops are fused
1024 wide. sin/cos tiles are bf16 (the c_r*w_h weighting is folded
into the q-side, keeping per-term magnitudes small). Inputs arrive as
ONE packed DRAM parameter (fewer PJRT buffer binds per dispatch).

Score accumulation note: matmul start=True clears the has_written bits
of the whole PSUM bank, so interleaved accumulation groups sharing a
bank cannot use it. A dummy full-bank start=True matmul writes zeros
and sets every bit; the real matmuls then accumulate with start=False
in r-streaming order. Scores for the two query halves go through the
same 4-bank PSUM region sequentially (8 banks total: 4 scores + 2x2
output/denominator); the tile framework's WAR tracking makes half B's
zeroing wait for half A's exp reads, and the PE fills that bubble with
half A's attn@V matmuls.

Softmax needs no max-subtraction (scores are O(1) by construction: w_v
has variance 1/H). exp runs once over the transposed scores [k, q] in
PSUM so the attn tile is directly the stationary operand of the attn@V
matmuls; the softmax denominator comes from one extra accumulating
matmul against a ones vector, followed by a DVE reciprocal and a
per-partition rescale of the output.

Dispatch-path notes (the dominant cost): the runner compiles under
bass2jax._fast_dispatch_active(True) so _bass_exec_p declares no
effect and calls take jax's C++ pjit fast path (the Python
effects/token path costs ~0.3-0.7 ms/call more); it returns the raw
Compiled rather than the FastDispatchCompiled wrapper (whose per-call
Python re-registration of every output shard costs ~0.1-0.2 ms); and
it passes no zero-filled output operands (on the exec lowering path
the NEFF binds only BIR ExternalInputs, and this kernel DMA-writes
every element of out). partition_id is disabled (unused input).
"""


import os
import numpy as np

import concourse.bass as bass
import concourse.mybir as mybir
import concourse.tile as tile
from concourse import bacc
from concourse.bass_utils import run_bass_kernel_spmd
from concourse.masks import make_identity

B, Q, K, D, H = 4, 512, 1024, 512, 128
N_CORES = 4
QSH = Q                         # 512 query rows per core (one batch/core)
QH = 256                        # query rows per PSUM score pass
NDC = D // 128                  # 4 contraction chunks
NKC = K // 128                  # 8 key chunks
NQB = QSH // 128                # 4 query blocks per core

F32 = mybir.dt.float32
BF16 = mybir.dt.bfloat16
EXP = mybir.ActivationFunctionType.Exp
SIN = mybir.ActivationFunctionType.Sin
COPY = mybir.ActivationFunctionType.Copy
TS = mybir.AluOpType

MAGIC = 12582912.0              # 1.5 * 2**23: fp32 add forces round-to-int
TWO_PI = float(2.0 * np.pi)

LAST_EXEC_NS = None
_NC_CACHE = {}


R_SINE = 8
WMAX = 4.5

def _fit_sine(R=R_SINE, wmax=WMAX, L=8.5, sigma=1.7):
    """Least-squares fit tanh(x) ~ sum_r c_r sin(w_r x) on [-L, L]."""
    ws = np.linspace(wmax / R * 0.5, wmax, R)
    xs = np.linspace(-L, L, 4001)
    wt = np.exp(-xs ** 2 / (2 * sigma ** 2)) + 1e-3
    A = np.sin(np.outer(xs, ws))
    Wt = np.sqrt(wt)[:, None]
    c, *_ = np.linalg.lstsq(A * Wt, np.tanh(xs) * Wt[:, 0], rcond=None)
    return [float(w) for w in ws], [float(v) for v in c]


# Packed single-input layout (one NEFF parameter instead of six: fewer PJRT
# buffer binds per dispatch, which dominates the per-call overhead under axon).
OFF_Q = 0
OFF_K = OFF_Q + QSH * D          # 262144
OFF_V = OFF_K + K * D            # 786432
OFF_WQ = OFF_V + K * D           # 1310720
OFF_WK = OFF_WQ + D * H          # 1376256
OFF_WV = OFF_WK + D * H          # 1441792
NPACK = OFF_WV + H               # 1441920


def _declare_io(nc):
    xin = nc.declare_dram_parameter("xin", [NPACK], F32, isOutput=False)
    out_ext = nc.declare_dram_parameter("out", [QSH, D], F32, isOutput=True)
    q_ext = xin[OFF_Q:OFF_K].rearrange("(q d) -> q d", d=D)
    k_ext = xin[OFF_K:OFF_V].rearrange("(k d) -> k d", d=D)
    v_ext = xin[OFF_V:OFF_WQ].rearrange("(k d) -> k d", d=D)
    wq_ext = xin[OFF_WQ:OFF_WK].rearrange("(d h) -> d h", h=H)
    wk_ext = xin[OFF_WK:OFF_WV].rearrange("(d h) -> d h", h=H)
    wv_ext = xin[OFF_WV:NPACK].rearrange("(h o) -> h o", o=1)
    return q_ext, k_ext, v_ext, wq_ext, wk_ext, wv_ext, out_ext


def _preamble(nc, tc, const, work, feat, q_ext, k_ext, v_ext, wq_ext, wk_ext,
              wv_ext):
    """DMA + PE-transpose inputs, feature matmuls, values cast.

    DMA instructions are BATCHED (one strided DMA per tensor / k-half
    instead of one per 128-row tile): each DMACopy costs ~0.6-1.2 us of
    sequencer issue plus ~0.9 us of completion-semaphore propagation, so
    27 small DMAs were a two-digit-us tax on the SEQ pipelines. All loads
    go on the SP (sync) queue; the ACT queue issues no DMAs so its
    sequencer bandwidth stays on activations. Keys load FIRST (in two
    512-row halves so the kf features and the k-side trig -- the long
    pole -- start as early as possible), then queries, then values
    (deferred: nothing reads v until the attn@V tail).

    Returns (fq [H, QSH+K] f32 ([qf | kf]), v_b [128, NKC, D] bf16,
    wv_f [H,1] f32)."""
    ident = const.tile([128, 128], F32)
    make_identity(nc, ident)

    wq_t = const.tile([128, NDC, H], F32)
    wk_t = const.tile([128, NDC, H], F32)
    nc.sync.dma_start(out=wk_t, in_=wk_ext.rearrange("(c p) h -> p c h", p=128))
    nc.sync.dma_start(out=wq_t, in_=wq_ext.rearrange("(c p) h -> p c h", p=128))

    wv_f = const.tile([H, 1], F32)
    nc.sync.dma_start(out=wv_f, in_=wv_ext[:])

    k_all = work.tile([128, NKC, D], F32, tag="kv")
    for hf in range(2):
        nc.sync.dma_start(
            out=k_all[:, hf * 4:(hf + 1) * 4, :],
            in_=k_ext[hf * 512:(hf + 1) * 512, :].rearrange(
                "(c p) d -> p c d", p=128))
    q_all = work.tile([128, QSH // 128, D], F32)
    nc.sync.dma_start(out=q_all,
                      in_=q_ext.rearrange("(c p) d -> p c d", p=128))

    qT = work.tile([128, NDC, QSH], F32)
    kT = work.tile([128, NDC, K], F32)
    # fq = [qf | kf] in ONE tile so the per-r trig argument prep can run as
    # single wide DVE ops over both sides at once.
    fq = feat.tile([H, QSH + K], F32)
    qf_sb = fq[:, :QSH]
    kf_sb = fq[:, QSH:]
    # Transposed copies go through a [128, NDC, 128] PSUM tile per source
    # tile: 4 PE transposes, then ONE wide PSUM->SBUF copy (the 48 narrow
    # copies' instruction + semaphore overhead was a preamble bottleneck).
    # kT/qT keep [dc][t] layout, so the 4 transposes of source tile t land
    # in one copy only if the copy dst is the [128, NDC, 128] column t --
    # strided but a single instruction.
    with tc.tile_pool(name="pre_ps", bufs=4, space="PSUM") as pre_ps:
        for t in range(K // 128):
            tp = pre_ps.tile([128, NDC, 128], F32, tag="tps")
            for dc in range(NDC):
                nc.tensor.transpose(tp[:, dc, :],
                                    k_all[:, t, dc * 128:(dc + 1) * 128],
                                    ident)
            dst = kT.rearrange("p c (t f) -> p c t f", f=128)[:, :, t, :]
            (nc.vector.tensor_copy(dst, tp) if t % 2 == 0
             else nc.scalar.copy(dst, tp))
            if t == 3 or t == 7:
                hf = t // 4
                kf_ps = pre_ps.tile([H, 512], F32, tag="fps")
                for dc in range(NDC):
                    nc.tensor.matmul(kf_ps, wk_t[:, dc, :],
                                     kT[:, dc, hf * 512:(hf + 1) * 512],
                                     start=(dc == 0), stop=(dc == NDC - 1))
                dst = kf_sb[:, hf * 512:(hf + 1) * 512]
                (nc.vector.tensor_copy(dst, kf_ps) if hf == 0
                 else nc.scalar.copy(dst, kf_ps))

        for t in range(QSH // 128):
            tp = pre_ps.tile([128, NDC, 128], F32, tag="tps")
            for dc in range(NDC):
                nc.tensor.transpose(tp[:, dc, :],
                                    q_all[:, t, dc * 128:(dc + 1) * 128],
                                    ident)
            dst = qT.rearrange("p c (t f) -> p c t f", f=128)[:, :, t, :]
            (nc.vector.tensor_copy(dst, tp) if t % 2 == 0
             else nc.scalar.copy(dst, tp))

        qf_ps = pre_ps.tile([H, QSH], F32, tag="fps")
        for dc in range(NDC):
            nc.tensor.matmul(qf_ps, wq_t[:, dc, :], qT[:, dc, :],
                             start=(dc == 0), stop=(dc == NDC - 1))
        nc.vector.tensor_copy(qf_sb, qf_ps)

    v_b = feat.tile([128, NKC, D], BF16)
    tc.tile_set_cur_wait(0.05)   # keep values off the keys->kf critical path
    # same tag as k_all: v reuses its buffer once the k transposes are done
    v_stage = work.tile([128, NKC, D], F32, tag="kv")
    nc.sync.dma_start(out=v_stage,
                      in_=v_ext.rearrange("(c p) d -> p c d", p=128))
    for hf in range(2):
        (nc.vector.tensor_copy if hf else nc.scalar.copy)(
            v_b[:, hf * 4:(hf + 1) * 4, :], v_stage[:, hf * 4:(hf + 1) * 4, :])
    tc.tile_set_cur_wait(0)

    return fq, v_b, wv_f


def _build_sine():
    ws, cs = _fit_sine()
    R = len(ws)
    nc = bacc.Bacc(enable_partition_id=False)
    q_ext, k_ext, v_ext, wq_ext, wk_ext, wv_ext, out_ext = _declare_io(nc)

    with tile.TileContext(nc) as tc:
        with tc.tile_pool(name="const", bufs=1) as const, \
             tc.tile_pool(name="feat", bufs=1) as feat, \
             tc.tile_pool(name="trig", bufs=3) as trig, \
             tc.tile_pool(name="oloop", bufs=2) as oloop:

            # The staging tiles (k_all/q_all/kT/qT/v_stage, ~64 KiB/
            # partition) live in their own pool that closes after the
            # preamble, freeing the space for the deeper (bufs=3) trig
            # pipeline.
            with tc.tile_pool(name="featpre", bufs=1) as featpre:
                fq, v_b, wv_f = _preamble(
                    nc, tc, const, featpre, feat, q_ext, k_ext, v_ext,
                    wq_ext, wk_ext, wv_ext)

            # per-r q-side coefficient vectors: wc[:, r] = c_r * w_v
            wc = const.tile([H, R], F32)
            for r in range(R):
                nc.vector.tensor_scalar_mul(wc[:, r:r + 1], wv_f, float(cs[r]))

            W2 = QSH + K                 # one trig block: [q(512) | k(1024)]
            # SCB[r] = [wc*sin_q(512) | sin_k(1024) |
            #           wc*cos_q(512) | cos_k(1024)]
            # One tile PER r (not [H, R, 2*W2]): the PE matmul AP encodes a
            # per-partition stride that maxes out at 16 KiB, and the fused
            # tile's 24 KiB row fails walrus's s3d3_mm ISA check.
            SCB = [feat.tile([H, 2 * W2], BF16, name=f"scb{r}")
                   for r in range(R)]

            # Per r, the range reduction is 3 wide DVE ops + 1 ACT round +
            # ONE double-wide ACT Sin over the combined [qf | kf] block:
            #   t   = x * (w/2pi)                  (DVE tensor_scalar)
            #   a   = fl(t + 1.5*2^23)             (ACT Copy, float bias:
            #                                       the fp32 store rounds t
            #                                       to the nearest integer)
            #   e_s = (a - MAGIC) - t              (DVE scalar_tensor_tensor,
            #                                       exact) = round(t) - t
            #   e_c = wrap(e_s - 1/4)              (custom-DVE range wrap
            #                                       into [-1/2, 1/2])
            #   sin/cos(w x) = Sin(-2pi * e)       (ONE ACT Sin over the
            #                                       contiguous [e_s | e_c]
            #                                       tile; table is accurate
            #                                       on [-pi, pi])
            # The q-side c_r*w_v weighting is applied in place on the bf16
            # sin/cos q slices (2 narrow DVE muls). No GPSIMD (Q7) anywhere:
            # those cost ~300us of per-dispatch host overhead under axon.
            for r in range(R):
                w2p = float(ws[r] / TWO_PI)
                wcol = wc[:, r:r + 1]
                t_t = trig.tile([H, W2], F32, tag="t")
                nc.vector.tensor_scalar(t_t, fq, w2p, None, TS.mult)
                a_t = trig.tile([H, W2], F32, tag="a")
                nc.scalar.activation(out=a_t, in_=t_t, func=COPY, bias=MAGIC)
                arg = trig.tile([H, 2 * W2], F32, tag="arg")
                nc.vector.scalar_tensor_tensor(arg[:, :W2], a_t, MAGIC, t_t,
                                               TS.subtract, TS.subtract)
                nc.vector.add_range_wrap(arg[:, W2:], arg[:, :W2],
                                         -0.25, 0.5, 1.0)
                nc.scalar.activation(out=SCB[r][:], in_=arg,
                                     func=SIN, scale=-TWO_PI)
                nc.vector.tensor_scalar_mul(SCB[r][:, 0:QSH],
                                            SCB[r][:, 0:QSH], wcol)
                nc.vector.tensor_scalar_mul(SCB[r][:, W2:W2 + QSH],
                                            SCB[r][:, W2:W2 + QSH], wcol)


            # Scores in [q, k] orientation: per 128-query block, stationary
            # = QS/QC q-chunk [H, 128] and moving = the FULL 1024-wide
            # KS/KC row, so one PSUM pass is 2R=16 matmuls of 1024 moving
            # cols instead of 128 matmuls of 256 cols. This unloads the PE
            # SEQUENCER (Ldweights+Matmult issue was the critical path at
            # 256 score matmuls). Each [128, 1024] f32 region is exactly 2
            # PSUM banks used by a single accumulation group, so plain
            # start/stop works -- no dummy-zero matmuls.
            #
            # attn is then transposed for the attn@V matmuls by ONE xbar
            # DMA-transpose per block (64 16x128 tiles, ~1 us on the idle
            # DMA engines) instead of 8 PE transposes + 8 PSUM copies. The
            # xbar writes logical row r of attn^T to attnT[r % 128,
            # r // 128, :] (hardware-verified), i.e. k-chunk c holds k rows
            # {c*128+p} -- the natural chunk layout v_b is loaded in.
            # Score matmuls for block qb+1 are issued BEFORE block qb's
            # transpose/AV so the PE never stalls on qb's exp.
            o_all = feat.tile([128, NQB, D], F32)
            with tc.tile_pool(name="psqk", bufs=3, space="PSUM") as psqk, \
                 tc.tile_pool(name="ps", bufs=2, space="PSUM") as ps:
                qsc = [None] * NQB

                def issue_scores(qb):
                    qs_sl = slice(qb * 128, (qb + 1) * 128)
                    qc_sl = slice(W2 + qb * 128, W2 + (qb + 1) * 128)
                    ks_sl = slice(QSH, W2)
                    kc_sl = slice(W2 + QSH, 2 * W2)
                    qsc[qb] = psqk.tile([128, K], F32, tag="qsc",
                                        name=f"qsc{qb}")
                    # moving operands are split into 512-wide halves: a
                    # 1024-element moving fmap fails walrus's s3d3_mm ISA
                    # check. The stationary is identical for both halves, so
                    # the second matmul skips its Ldweights.
                    for r in range(R):
                        for hf in range(2):
                            osl = slice(hf * 512, (hf + 1) * 512)
                            nc.tensor.matmul(
                                qsc[qb][:, osl], SCB[r][:, qs_sl],
                                SCB[r][:, kc_sl][:, osl],
                                start=(r == 0), stop=False,
                                skip_group_check=True)
                        for hf in range(2):
                            osl = slice(hf * 512, (hf + 1) * 512)
                            nc.tensor.matmul(
                                qsc[qb][:, osl], SCB[r][:, qc_sl],
                                SCB[r][:, ks_sl][:, osl],
                                start=False, stop=(r == R - 1),
                                skip_group_check=True)

                issue_scores(0)
                issue_scores(1)
                for qb in range(NQB):
                    attnQ = oloop.tile([128, K], BF16, tag="attnQ")
                    d_sb = oloop.tile([128, 1], F32, tag="dsb")
                    # accum_out gives the softmax denominator for free: in
                    # the [q, k] orientation the activation's per-partition
                    # output sum IS sum_k exp(score[q, k]).
                    nc.scalar.activation(out=attnQ, in_=qsc[qb], func=EXP,
                                         accum_out=d_sb)
                    if qb + 2 < NQB:
                        issue_scores(qb + 2)
                    attnT = oloop.tile([128, NKC, 128], BF16, tag="attnT")
                    nc.sync.dma_start_transpose(out=attnT[:], in_=attnQ[:])
                    o_ps = ps.tile([128, D], F32, tag="ops")
                    for kc in range(NKC):
                        nc.tensor.matmul(o_ps, attnT[:, kc, :], v_b[:, kc, :],
                                         start=(kc == 0), stop=(kc == NKC - 1))
                    recip = oloop.tile([128, 1], F32, tag="recip")
                    nc.vector.reciprocal(recip, d_sb)
                    nc.vector.tensor_scalar_mul(o_all[:, qb, :], o_ps, recip)
                nc.sync.dma_start(
                    out=out_ext.rearrange("(t p) d -> p t d", p=128),
                    in_=o_all)
    nc.compile()
    return nc


def _get_nc():
    if "sine" not in _NC_CACHE:
        _NC_CACHE["sine"] = _build_sine()
    return _NC_CACHE["sine"]


def make_in_maps(queries, keys, values, W_q, W_k, w_v):
    queries = np.asarray(queries, dtype=np.float32)
    keys = np.asarray(keys, dtype=np.float32)
    values = np.asarray(values, dtype=np.float32)
    W_q = np.asarray(W_q, dtype=np.float32).ravel()
    W_k = np.asarray(W_k, dtype=np.float32).ravel()
    w_v = np.asarray(w_v, dtype=np.float32).ravel()
    in_maps = []
    for c in range(N_CORES):
        buf = np.empty(NPACK, np.float32)
        buf[OFF_Q:OFF_K] = queries[c].ravel()
        buf[OFF_K:OFF_V] = keys[c].ravel()
        buf[OFF_V:OFF_WQ] = values[c].ravel()
        buf[OFF_WQ:OFF_WK] = W_q
        buf[OFF_WK:OFF_WV] = W_k
        buf[OFF_WV:NPACK] = w_v
        in_maps.append({"xin": buf})
    return in_maps


_RUNNER_CACHE = {}


def _get_runner(nc):
    """Persistent compiled shard_map runner for nc (compiled once/process).

    Two dispatch-path choices matter for the marginal per-call cost under
    axon (the per-dispatch host overhead dominates on-device time):

    * compile under bass2jax._fast_dispatch_active(True): _bass_exec_p then
      declares no effect, so calls take jax's C++ pjit fast path instead of
      the Python effects/token dispatch (~0.3-0.7 ms/call cheaper).
    * return the raw Compiled, NOT FastDispatchCompiled: the safety-net
      wrapper re-registers every output shard in runtime_tokens on every
      call (a Python loop over the shards, ~0.1-0.2 ms/call). kernel()
      reads its outputs immediately, so device errors surface regardless.
    * no zero-filled output operands: on the exec lowering path the NEFF
      binds only BIR ExternalInputs (the "out" zeros param has no NEFF
      tensor and is ignored), and this kernel DMA-writes every element of
      out, so PJRT's uninitialized result allocation is fine. Dropping
      them saves one buffer bind per core per call.
    """
    if id(nc) in _RUNNER_CACHE:
        return _RUNNER_CACHE[id(nc)]
    import jax
    from jax.sharding import Mesh, NamedSharding, PartitionSpec
    from jax.experimental.shard_map import shard_map
    from concourse import bass2jax

    bass2jax.install_neuronx_cc_hook()
    partition_name = (nc.partition_id_tensor.name
                      if nc.partition_id_tensor else None)
    in_names, in_shapes, out_names, out_avals = [], [], [], []
    for alloc in nc.m.functions[0].allocations:
        if not isinstance(alloc, mybir.MemoryLocationSet):
            continue
        name = alloc.memorylocations[0].name
        if alloc.kind == "ExternalInput":
            if name != partition_name:
                in_names.append(name)
                in_shapes.append(
                    (tuple(alloc.tensor_shape), mybir.dt.np(alloc.dtype)))
        elif alloc.kind == "ExternalOutput":
            out_names.append(name)
            shape = tuple(alloc.tensor_shape)
            dtype = mybir.dt.np(alloc.dtype)
            out_avals.append(jax.core.ShapedArray(shape, dtype))
    all_in_names = list(in_names)
    if partition_name is not None:
        all_in_names.append(partition_name)

    def _body(*args):
        operands = list(args)
        if partition_name is not None:
            operands.append(bass2jax.partition_id_tensor())
        outs = bass2jax._bass_exec_p.bind(
            *operands,
            out_avals=tuple(out_avals),
            in_names=tuple(all_in_names),
            out_names=tuple(out_names),
            lowering_input_output_aliases=(),
            sim_require_finite=True,
            sim_require_nnan=True,
            nc=nc,
        )
        return tuple(outs)

    devices = jax.devices()[:N_CORES]
    mesh = Mesh(np.asarray(devices), ("core",))
    nio = len(in_names)
    sharding = NamedSharding(mesh, PartitionSpec("core"))
    fast_ctx = getattr(bass2jax, "_fast_dispatch_active", None)
    import contextlib
    with (fast_ctx(True) if fast_ctx is not None
          else contextlib.nullcontext()):
        f = jax.jit(
            shard_map(_body, mesh=mesh,
                      in_specs=(PartitionSpec("core"),) * nio,
                      out_specs=(PartitionSpec("core"),) * len(out_names),
                      check_rep=False),
            keep_unused=True,
        )
        dummy_in = [
            jax.ShapeDtypeStruct((N_CORES * shape[0], *shape[1:]), dtype,
                                 sharding=sharding)
            for shape, dtype in in_shapes
        ]
        fc = f.lower(*dummy_in).compile()
    runner = (fc, in_names, out_names, out_avals, sharding)
    _RUNNER_CACHE[id(nc)] = runner
    return runner


def kernel(queries, keys, values, W_q, W_k, w_v):
    import jax
    nc = _get_nc()
    in_maps = make_in_maps(queries, keys, values, W_q, W_k, w_v)
    try:
        fc, in_names, out_names, out_avals, sharding = _get_runner(nc)
        concat_in = [
            np.concatenate([in_maps[c][name] for c in range(N_CORES)], axis=0)
            for name in in_names
        ]
        args = [jax.device_put(a, sharding) for a in concat_in]
        out_arrs = fc(*args)
        results = [
            {name: np.asarray(out_arrs[i]).reshape(
                N_CORES, *out_avals[i].shape)[c]
             for i, name in enumerate(out_names)}
            for c in range(N_CORES)
        ]
    except Exception:
        res = run_bass_kernel_spmd(nc, in_maps, core_ids=list(range(N_CORES)))
        results = res.results

    out = np.empty((B, Q, D), dtype=np.float32)
    for c in range(N_CORES):
        out[c] = results[c]["out"]
    return out
